# revision 3
# baseline (speedup 1.0000x reference)
"""Multi-head attention (B=4, S=2048, E=1024, H=16, causal) on 8 Trainium2 cores.

Sharding: core = (batch b, head-group g) — 4 batches x 2 groups of 8 heads.
Each core computes q/k/v projections for its batch restricted to its 8 heads,
causal attention for those heads, and a partial output projection over its
512 ctx columns.  The host sums the two partials per batch and adds all
output-side bias terms analytically (softmax rows sum to one, so the v-bias
passes through attention unchanged: out += o_b + v_b @ o_w.T).

On-device layouts (per core):
  qT/kT: [head_dim 512 -> 4 tiles of 128, token 2048]  (2 heads per tile)
  v_aug: [token -> 16 tiles of 128, 8 heads x (64 dims + ones col)]
  scores are computed transposed (k^T q per head, contraction dim 64,
  two heads row-tiled concurrently in the PE array), softmax is max-free
  (scores are O(+-8), exp cannot overflow fp32), causal masking is
  structural: fully-masked tiles are skipped, diagonal tiles get a
  memset + affine_select staircase fill.
  attn @ v is computed as v_aug^T @ expT giving ctx^T plus the softmax
  row-sum in one matmul (ones column of v_aug).
"""

import os
import sys

for _p in ("/opt/trn_rl_repo", "/root/.axon_site/_ro/trn_rl_repo"):
    if os.path.isdir(_p) and _p not in sys.path:
        sys.path.append(_p)

import numpy as np
import ml_dtypes

import concourse.bacc as bacc
import concourse.mybir as mybir
from concourse import tile
from concourse import bass_utils
from concourse.bass import ts

BF16 = ml_dtypes.bfloat16
F32 = mybir.dt.float32
BF = mybir.dt.bfloat16
AFT = mybir.ActivationFunctionType

B, S, E = 4, 2048, 1024
H, D = 16, 64
G = 512            # head dims per core (8 heads)
KC = E // 128      # contraction chunks for projections
NM = G // 128      # m-tiles of the group dim
NJ = S // 512      # 512-wide token column blocks
NT = S // 128      # 128-wide token tiles

_NC = None


def _build():
    nc = bacc.Bacc("TRN2", target_bir_lowering=False, debug=False, num_devices=8)

    xq = nc.dram_tensor("xq", (E, S), BF, kind="ExternalInput").ap()
    xk = nc.dram_tensor("xk", (E, S), BF, kind="ExternalInput").ap()
    xv = nc.dram_tensor("xv", (E, S), BF, kind="ExternalInput").ap()
    wq = nc.dram_tensor("wq", (E, G), BF, kind="ExternalInput").ap()
    wk = nc.dram_tensor("wk", (E, G), BF, kind="ExternalInput").ap()
    wv = nc.dram_tensor("wv", (E, G), BF, kind="ExternalInput").ap()
    wo = nc.dram_tensor("wo", (G, E), BF, kind="ExternalInput").ap()
    qb = nc.dram_tensor("qb", (128, NM), F32, kind="ExternalInput").ap()
    kb = nc.dram_tensor("kb", (128, NM), F32, kind="ExternalInput").ap()
    fT = nc.dram_tensor("fT", (E, S), F32, kind="ExternalOutput").ap()

    with tile.TileContext(nc) as tc:
        with (
            tc.tile_pool(name="cst", bufs=2) as cst,
            tc.tile_pool(name="wsb", bufs=12) as wsb,
            tc.tile_pool(name="xs", bufs=16) as xsp,
            tc.tile_pool(name="qt", bufs=8) as qtp,
            tc.tile_pool(name="va", bufs=16) as vap,
            tc.tile_pool(name="ctx", bufs=4) as ctxp,
            tc.tile_pool(name="exp", bufs=12) as expp,
            tc.tile_pool(name="wo", bufs=4) as wop,
            tc.tile_pool(name="fin", bufs=4) as finp,
            tc.tile_pool(name="sm", bufs=6) as smp,
            tc.tile_pool(name="rb", bufs=4) as rbp,
            tc.tile_pool(name="tmp", bufs=4) as tmpp,
            tc.tile_pool(name="pss", bufs=4, space="PSUM") as pss,
            tc.tile_pool(name="psc", bufs=4, space="PSUM") as psc,
        ):
            qb_t = cst.tile([128, NM], F32, tag="cst")
            nc.sync.dma_start(qb_t[:, :], qb[:, :])
            kb_t = cst.tile([128, NM], F32, tag="cst")
            nc.sync.dma_start(kb_t[:, :], kb[:, :])

            zero_fill = nc.gpsimd.to_reg(0.0)

            qT = [qtp.tile([128, S], BF, tag="qt", name=f"qT{m}") for m in range(NM)]
            kT = [qtp.tile([128, S], BF, tag="qt", name=f"kT{m}") for m in range(NM)]
            ctxT = [ctxp.tile([128, S], BF, tag="ctx", name=f"ctxT{m}") for m in range(NM)]

            # ---- q / k projections: out = w^T x  (qT layout [dim, token]) --
            for x_ap, w_ap, dst, bias_t, scale in (
                (xq, wq, qT, qb_t, 0.125),
                (xk, wk, kT, kb_t, 1.0),
            ):
                w_sb = [wsb.tile([128, G], BF, tag="w", name=f"w{kc}") for kc in range(KC)]
                for kc in range(KC):
                    nc.sync.dma_start(w_sb[kc][:, :], w_ap[ts(kc, 128), :])
                for n in range(NJ):
                    xs = [xsp.tile([128, 512], BF, tag="xs", name=f"xs{kc}") for kc in range(KC)]
                    for kc in range(KC):
                        nc.sync.dma_start(
                            xs[kc][:, :], x_ap[ts(kc, 128), ts(n, 512)]
                        )
                    for m in range(NM):
                        ps = pss.tile([128, 512], F32, tag="ps")
                        for kc in range(KC):
                            nc.tensor.matmul(
                                ps[:, :],
                                w_sb[kc][:, ts(m, 128)],
                                xs[kc][:, :],
                                start=(kc == 0),
                                stop=(kc == KC - 1),
                            )
                        nc.scalar.activation(
                            dst[m][:, ts(n, 512)],
                            ps[:, :],
                            AFT.Identity,
                            bias=bias_t[:, m : m + 1],
                            scale=scale,
                        )

            # ---- v projection: out = x^T w  ([token, head x (64+ones)]) ----
            v_aug = []
            w_sb = [wsb.tile([128, G], BF, tag="w", name=f"wv{kc}") for kc in range(KC)]
            for kc in range(KC):
                nc.sync.dma_start(w_sb[kc][:, :], wv[ts(kc, 128), :])
            for tg in range(NJ):
                xs = [xsp.tile([128, 512], BF, tag="xs", name=f"xsv{kc}") for kc in range(KC)]
                for kc in range(KC):
                    nc.sync.dma_start(xs[kc][:, :], xv[ts(kc, 128), ts(tg, 512)])
                for t4 in range(4):
                    ps = pss.tile([128, 512], F32, tag="ps")
                    for kc in range(KC):
                        nc.tensor.matmul(
                            ps[:, :],
                            xs[kc][:, ts(t4, 128)],
                            w_sb[kc][:, :],
                            start=(kc == 0),
                            stop=(kc == KC - 1),
                        )
                    va = vap.tile([128, 8 * 65], BF, tag="va")
                    va3 = va[:, :].rearrange("p (h x) -> p h x", h=8)
                    ps3 = ps[:, :].rearrange("p (h x) -> p h x", h=8)
                    nc.scalar.activation(va3[:, :, 0:64], ps3[:, :, :], AFT.Copy)
                    nc.vector.memset(va3[:, :, 64:65], 1.0)
                    v_aug.append(va)

            # ---- attention, head pairs row-tiled in the PE array ----------
            for hp in range(NM):
                for j in range(NJ):
                    ni = 4 * j + 4  # causal: tk tiles 0..4j+3
                    cA = psc.tile([65, 512], F32, tag="psc")
                    cB = psc.tile([65, 512], F32, tag="psc")
                    for i in range(ni):
                        r = i - 4 * j  # >=0 on diagonal-crossing tiles
                        sA = pss.tile([128, 512], F32, tag="ps")
                        sB = pss.tile([128, 512], F32, tag="ps")
                        nc.tensor.matmul(
                            sA[:, :],
                            kT[hp][0:64, ts(i, 128)],
                            qT[hp][0:64, ts(j, 512)],
                            start=True,
                            stop=True,
                        )
                        nc.tensor.matmul(
                            sB[:, :],
                            kT[hp][64:128, ts(i, 128)],
                            qT[hp][64:128, ts(j, 512)],
                            start=True,
                            stop=True,
                            tile_position=(64, 0),
                        )
                        eA = expp.tile([128, 512], BF, tag="exp")
                        eB = expp.tile([128, 512], BF, tag="exp")
                        for s, e in ((sA, eA), (sB, eB)):
                            if r < 0:
                                nc.scalar.activation(e[:, :], s[:, :], AFT.Exp)
                            else:
                                if r > 0:
                                    nc.vector.memset(e[:, 0 : 128 * r], 0.0)
                                nc.scalar.activation(
                                    e[:, 128 * r : 512], s[:, 128 * r : 512], AFT.Exp
                                )
                                # keep exp where col >= row within the 128x128
                                # diagonal block, else 0
                                nc.gpsimd.affine_select(
                                    out=e[:, 128 * r : 128 * (r + 1)],
                                    in_=e[:, 128 * r : 128 * (r + 1)],
                                    pattern=[[1, 128]],
                                    compare_op=mybir.AluOpType.is_ge,
                                    fill=zero_fill,
                                    base=0,
                                    channel_multiplier=-1,
                                )
                        hA, hB = 2 * hp, 2 * hp + 1
                        nc.tensor.matmul(
                            cA[:, :],
                            v_aug[i][:, hA * 65 : (hA + 1) * 65],
                            eA[:, :],
                            start=(i == 0),
                            stop=(i == ni - 1),
                        )
                        nc.tensor.matmul(
                            cB[:, :],
                            v_aug[i][:, hB * 65 : (hB + 1) * 65],
                            eB[:, :],
                            start=(i == 0),
                            stop=(i == ni - 1),
                        )
                    # normalize by the softmax row-sum (row 64 of cA/cB)
                    for c, half in ((cA, 0), (cB, 1)):
                        rec = smp.tile([1, 512], F32, tag="sm")
                        nc.vector.reciprocal(rec[:, :], c[64:65, :])
                        rb = rbp.tile([64, 512], F32, tag="rb")
                        nc.gpsimd.partition_broadcast(rb[:, :], rec[:, :])
                        if half == 0:
                            nc.vector.tensor_mul(
                                ctxT[hp][0:64, ts(j, 512)], c[0:64, :], rb[:, :]
                            )
                        else:
                            tm = tmpp.tile([64, 512], BF, tag="tmp")
                            nc.vector.tensor_mul(tm[:, :], c[0:64, :], rb[:, :])
                            nc.sync.dma_start(
                                ctxT[hp][64:128, ts(j, 512)], tm[:, :]
                            )

            # ---- output projection (partial over this core's 512 dims) ----
            wo_sb = [wop.tile([128, E], BF, tag="wo", name=f"wo{ec}") for ec in range(NM)]
            for ec in range(NM):
                nc.sync.dma_start(wo_sb[ec][:, :], wo[ts(ec, 128), :])
            for q4 in range(NJ):
                for jt in range(E // 128):
                    ps = pss.tile([128, 512], F32, tag="ps")
                    for ec in range(NM):
                        nc.tensor.matmul(
                            ps[:, :],
                            wo_sb[ec][:, ts(jt, 128)],
                            ctxT[ec][:, ts(q4, 512)],
                            start=(ec == 0),
                            stop=(ec == NM - 1),
                        )
                    st = finp.tile([128, 512], F32, tag="fin")
                    nc.scalar.activation(st[:, :], ps[:, :], AFT.Copy)
                    nc.sync.dma_start(fT[ts(jt, 128), ts(q4, 512)], st[:, :])

    nc.compile()
    return nc


def _get_nc():
    global _NC
    if _NC is None:
        _NC = _build()
    return _NC


def kernel(**inputs):
    query = np.asarray(inputs["query"], np.float32)
    key = np.asarray(inputs["key"], np.float32)
    value = np.asarray(inputs["value"], np.float32)
    q_w = np.asarray(inputs["q_w"], np.float32)
    q_b = np.asarray(inputs["q_b"], np.float32)
    k_w = np.asarray(inputs["k_w"], np.float32)
    k_b = np.asarray(inputs["k_b"], np.float32)
    v_w = np.asarray(inputs["v_w"], np.float32)
    v_b = np.asarray(inputs["v_b"], np.float32)
    o_w = np.asarray(inputs["o_w"], np.float32)
    o_b = np.asarray(inputs["o_b"], np.float32)

    nc = _get_nc()

    xqT = [np.ascontiguousarray(query[b].T).astype(BF16) for b in range(B)]
    xkT = [np.ascontiguousarray(key[b].T).astype(BF16) for b in range(B)]
    xvT = [np.ascontiguousarray(value[b].T).astype(BF16) for b in range(B)]

    wqT, wkT, wvT, woT, qbt, kbt = [], [], [], [], [], []
    for g in range(2):
        gs = slice(g * G, (g + 1) * G)
        wqT.append(np.ascontiguousarray(q_w[gs, :].T).astype(BF16))
        wkT.append(np.ascontiguousarray(k_w[gs, :].T).astype(BF16))
        wvT.append(np.ascontiguousarray(v_w[gs, :].T).astype(BF16))
        woT.append(np.ascontiguousarray(o_w[:, gs].T).astype(BF16))
        qbt.append(
            np.ascontiguousarray((q_b[gs] / 8.0).reshape(NM, 128).T).astype(
                np.float32
            )
        )
        kbt.append(
            np.ascontiguousarray(k_b[gs].reshape(NM, 128).T).astype(np.float32)
        )

    in_maps = []
    for b in range(B):
        for g in range(2):
            in_maps.append(
                {
                    "xq": xqT[b],
                    "xk": xkT[b],
                    "xv": xvT[b],
                    "wq": wqT[g],
                    "wk": wkT[g],
                    "wv": wvT[g],
                    "wo": woT[g],
                    "qb": qbt[g],
                    "kb": kbt[g],
                }
            )

    res = bass_utils.run_bass_kernel_spmd(nc, in_maps, core_ids=list(range(8)))

    corr = (o_b + v_b @ o_w.T).astype(np.float32)  # softmax rows sum to 1
    out = np.empty((B, S, E), np.float32)
    for b in range(B):
        acc = res.results[2 * b]["fT"] + res.results[2 * b + 1]["fT"]
        out[b] = acc.T + corr[None, :]
    return out


# revision 7
# speedup vs baseline: 1.5959x; 1.5959x over previous
"""Multi-head attention (B=4, S=2048, E=1024, H=16, causal) on 8 Trainium2 cores.

Sharding: core = (batch b, head-group g) — 4 batches x 2 groups of 8 heads.
Each core computes q/k/v projections for its batch restricted to its 8 heads,
causal attention for those heads, and a partial output projection over its
512 ctx columns.  The host sums the two partials per batch and adds all
output-side bias terms analytically (softmax rows sum to one, so the v-bias
passes through attention unchanged: out += o_b + v_b @ o_w.T).

On-device layouts (per core):
  qT/kT: [head_dim 512 -> 4 tiles of 128, token 2048]  (2 heads per tile)
  v_aug: [token -> 16 tiles of 128, 8 heads x (64 dims + ones col)]
  scores are computed transposed (k^T q per head, contraction dim 64,
  two heads row-tiled concurrently in the PE array), softmax is max-free
  (scores are O(+-8), exp cannot overflow fp32), causal masking is
  structural: fully-masked tiles are skipped, diagonal tiles get a
  memset + affine_select staircase fill.
  attn @ v is computed as v_aug^T @ expT giving ctx^T plus the softmax
  row-sum in one matmul (ones column of v_aug).

The program is software-pipelined over token column blocks n=0..3:
projections for block n feed the attention column block j=n and the
output projection for the same block, so the PE-heavy projection work of
block n+1 overlaps the ScalarE-heavy exp work of block n.  Score tiles
are paired into [128,1024] PSUM tensors so one exp instruction covers
two k-tiles (halves ScalarE per-op overhead).
"""

import os
import sys

for _p in ("/opt/trn_rl_repo", "/root/.axon_site/_ro/trn_rl_repo"):
    if os.path.isdir(_p) and _p not in sys.path:
        sys.path.append(_p)

import numpy as np
import ml_dtypes

import concourse.bacc as bacc
import concourse.mybir as mybir
from concourse import tile
from concourse import bass_utils
from concourse.bass import ts

BF16 = ml_dtypes.bfloat16
F32 = mybir.dt.float32
BF = mybir.dt.bfloat16
AFT = mybir.ActivationFunctionType
ALU = mybir.AluOpType

B, S, E = 4, 2048, 1024
H, D = 16, 64
G = 512            # head dims per core (8 heads)
KC = E // 128      # contraction chunks for projections
NM = G // 128      # m-tiles of the group dim
NJ = S // 512      # 512-wide token column blocks
NT = S // 128      # 128-wide token tiles

_NC = None


def _build():
    nc = bacc.Bacc("TRN2", target_bir_lowering=False, debug=False, num_devices=8)

    xq = nc.dram_tensor("xq", (E, S), BF, kind="ExternalInput").ap()
    xk = nc.dram_tensor("xk", (E, S), BF, kind="ExternalInput").ap()
    xv = nc.dram_tensor("xv", (E, S), BF, kind="ExternalInput").ap()
    wq = nc.dram_tensor("wq", (E, G), BF, kind="ExternalInput").ap()
    wk = nc.dram_tensor("wk", (E, G), BF, kind="ExternalInput").ap()
    wv = nc.dram_tensor("wv", (E, G), BF, kind="ExternalInput").ap()
    wo = nc.dram_tensor("wo", (G, E), BF, kind="ExternalInput").ap()
    qb = nc.dram_tensor("qb", (128, NM), F32, kind="ExternalInput").ap()
    kb = nc.dram_tensor("kb", (128, NM), F32, kind="ExternalInput").ap()
    sel = nc.dram_tensor("sel", (8, G), BF, kind="ExternalInput").ap()
    fT = nc.dram_tensor("fT", (E, S), F32, kind="ExternalOutput").ap()

    with tile.TileContext(nc) as tc:
        with (
            tc.tile_pool(name="cst", bufs=2) as cst,
            tc.tile_pool(name="wsb", bufs=24) as wsb,
            tc.tile_pool(name="xs", bufs=16) as xsp,
            tc.tile_pool(name="qt", bufs=8) as qtp,
            tc.tile_pool(name="va", bufs=16) as vap,
            tc.tile_pool(name="ctx", bufs=4) as ctxp,
            tc.tile_pool(name="exp", bufs=8) as expp,
            tc.tile_pool(name="wo", bufs=4) as wop,
            tc.tile_pool(name="fin", bufs=4) as finp,
            tc.tile_pool(name="sm", bufs=8) as smp,
            tc.tile_pool(name="rb", bufs=4) as rbp,
            tc.tile_pool(name="tmp", bufs=4) as tmpp,
            tc.tile_pool(name="ps", bufs=3, space="PSUM") as psp,
            tc.tile_pool(name="psc", bufs=2, space="PSUM") as pscp,
        ):
            qb_t = cst.tile([128, NM], F32, tag="cst")
            nc.sync.dma_start(qb_t[:, :], qb[:, :])
            kb_t = cst.tile([128, NM], F32, tag="cst")
            nc.sync.dma_start(kb_t[:, :], kb[:, :])

            zero_fill = nc.gpsimd.to_reg(0.0)

            # one-hot head-selector rows (partitions 64..71) for the
            # reciprocal-broadcast matmul
            sel_sb = cst.tile([72, G], BF, tag="sel", name="sel_sb")
            nc.sync.dma_start(sel_sb[64:72, :], sel[:, :])
            # constant ones-slots pattern for v_aug cols [64..72) per head
            ones_c = cst.tile([128, 8 * 72], BF, tag="ones", name="ones_c")
            ones_c3 = ones_c[:, :].rearrange("p (h x) -> p h x", h=8)
            nc.vector.memset(ones_c3[:, :, 64:72], 0.0)
            for h in range(8):
                nc.vector.memset(ones_c3[:, h : h + 1, 64 + h : 65 + h], 1.0)

            qT = [qtp.tile([128, S], BF, tag="qt", name=f"qT{m}") for m in range(NM)]
            kT = [qtp.tile([128, S], BF, tag="qt", name=f"kT{m}") for m in range(NM)]
            ctxT = [ctxp.tile([128, S], BF, tag="ctx", name=f"ctxT{m}")
                    for m in range(NM)]
            v_aug = [None] * NT

            # weights stay resident for the whole kernel
            wq_sb = [wsb.tile([128, G], BF, tag="w", name=f"wq{kc}") for kc in range(KC)]
            wk_sb = [wsb.tile([128, G], BF, tag="w", name=f"wk{kc}") for kc in range(KC)]
            wv_sb = [wsb.tile([128, G], BF, tag="w", name=f"wv{kc}") for kc in range(KC)]
            for kc in range(KC):
                nc.sync.dma_start(wq_sb[kc][:, :], wq[ts(kc, 128), :])
                nc.sync.dma_start(wk_sb[kc][:, :], wk[ts(kc, 128), :])
                nc.sync.dma_start(wv_sb[kc][:, :], wv[ts(kc, 128), :])
            wo_sb = [wop.tile([128, E], BF, tag="wo", name=f"wo{ec}") for ec in range(NM)]
            for ec in range(NM):
                nc.sync.dma_start(wo_sb[ec][:, :], wo[ts(ec, 128), :])

            def proj_qk(n, x_ap, w_sb, dst, bias_t, scale):
                xs = [xsp.tile([128, 512], BF, tag="xs", name=f"xs{kc}")
                      for kc in range(KC)]
                for kc in range(KC):
                    nc.sync.dma_start(xs[kc][:, :], x_ap[ts(kc, 128), ts(n, 512)])
                for mp in range(2):
                    psd = psp.tile([128, 1024], F32, tag="ps1024", name="psd")
                    ps0, ps1 = psd[:, 0:512], psd[:, 512:1024]
                    for kc in range(KC):
                        nc.tensor.matmul(
                            ps0, w_sb[kc][:, ts(2 * mp, 128)], xs[kc][:, :],
                            start=(kc == 0), stop=(kc == KC - 1))
                        nc.tensor.matmul(
                            ps1, w_sb[kc][:, ts(2 * mp + 1, 128)], xs[kc][:, :],
                            start=(kc == 0), stop=(kc == KC - 1))
                    for mh, ps in ((0, ps0), (1, ps1)):
                        m = 2 * mp + mh
                        nc.vector.tensor_scalar(
                            dst[m][:, ts(n, 512)], ps,
                            scale, bias_t[:, m : m + 1],
                            ALU.mult, ALU.add)

            def proj_v(tg):
                xs = [xsp.tile([128, 512], BF, tag="xs", name=f"xsv{kc}")
                      for kc in range(KC)]
                for kc in range(KC):
                    nc.sync.dma_start(xs[kc][:, :], xv[ts(kc, 128), ts(tg, 512)])
                for tp in range(2):
                    psd = psp.tile([128, 1024], F32, tag="ps1024", name="psd")
                    ps0, ps1 = psd[:, 0:512], psd[:, 512:1024]
                    for kc in range(KC):
                        nc.tensor.matmul(
                            ps0, xs[kc][:, ts(2 * tp, 128)], wv_sb[kc][:, :],
                            start=(kc == 0), stop=(kc == KC - 1))
                        nc.tensor.matmul(
                            ps1, xs[kc][:, ts(2 * tp + 1, 128)], wv_sb[kc][:, :],
                            start=(kc == 0), stop=(kc == KC - 1))
                    for th, ps in ((0, ps0), (1, ps1)):
                        tt = 4 * tg + 2 * tp + th
                        va = vap.tile([128, 8 * 72], BF, tag="va", name=f"va{tt}")
                        va3 = va[:, :].rearrange("p (h x) -> p h x", h=8)
                        ps3 = ps.rearrange("p (h x) -> p h x", h=8)
                        nc.vector.tensor_copy(va3[:, :, 0:64], ps3[:, :, :])
                        nc.vector.tensor_copy(
                            va3[:, :, 64:72], ones_c3[:, :, 64:72])
                        v_aug[tt] = va

            def attention_block(hp, j, rs_j):
                ni = 4 * j + 4          # causal: tk tiles 0..4j+3 (always even)
                np2 = ni // 2
                cA = pscp.tile([72, 512], F32, tag="psc", name="cA")
                cB = pscp.tile([72, 512], F32, tag="psc", name="cB")
                hA, hB = 2 * hp, 2 * hp + 1
                for ip in range(np2):
                    i0, i1 = 2 * ip, 2 * ip + 1
                    sA = psp.tile([128, 1024], F32, tag="ps1024", name="sA")
                    sB = psp.tile([128, 1024], F32, tag="ps1024", name="sB")
                    for half, i in ((0, i0), (1, i1)):
                        nc.tensor.matmul(
                            sA[:, ts(half, 512)],
                            kT[hp][0:64, ts(i, 128)], qT[hp][0:64, ts(j, 512)],
                            start=True, stop=True)
                        nc.tensor.matmul(
                            sB[:, ts(half, 512)],
                            kT[hp][64:128, ts(i, 128)], qT[hp][64:128, ts(j, 512)],
                            start=True, stop=True, tile_position=(64, 0))
                    eA = expp.tile([128, 1024], BF, tag="exp", name="eA")
                    eB = expp.tile([128, 1024], BF, tag="exp", name="eB")
                    nc.scalar.activation(eA[:, :], sA[:, :], AFT.Exp)
                    nc.scalar.activation(eB[:, :], sB[:, :], AFT.Exp)
                    for half, i in ((0, i0), (1, i1)):
                        r = i - 4 * j
                        if r < 0:
                            continue
                        for e in (eA, eB):
                            # zero everything left of / above the diagonal in
                            # one pass: keep iff col - 128r - row >= 0
                            nc.gpsimd.affine_select(
                                out=e[:, 512 * half : 512 * half + 128 * (r + 1)],
                                in_=e[:, 512 * half : 512 * half + 128 * (r + 1)],
                                pattern=[[1, 128 * (r + 1)]],
                                compare_op=ALU.is_ge,
                                fill=zero_fill,
                                base=-128 * r,
                                channel_multiplier=-1)
                    for half, i in ((0, i0), (1, i1)):
                        nc.tensor.matmul(
                            cA[:, :], v_aug[i][:, hA * 72 : hA * 72 + 72],
                            eA[:, ts(half, 512)],
                            start=(i == 0), stop=(i == ni - 1))
                        nc.tensor.matmul(
                            cB[:, :], v_aug[i][:, hB * 72 : hB * 72 + 72],
                            eB[:, ts(half, 512)],
                            start=(i == 0), stop=(i == ni - 1))
                # evacuate unnormalized ctx to SBUF; accumulate row-sums
                # (each head occupies its own partition in the 64..72 band)
                for c, half in ((cA, 0), (cB, 1)):
                    nc.vector.tensor_add(
                        rs_j[64:72, :], rs_j[64:72, :], c[64:72, :])
                    if half == 0:
                        nc.vector.tensor_copy(ctxT[hp][0:64, ts(j, 512)], c[0:64, :])
                    else:
                        tm = tmpp.tile([64, 512], BF, tag="tmp", name="tm")
                        nc.vector.tensor_copy(tm[:, :], c[0:64, :])
                        nc.sync.dma_start(ctxT[hp][64:128, ts(j, 512)], tm[:, :])

            def oproj_block(q4):
                for jtp in range(4):
                    psd = psp.tile([128, 1024], F32, tag="ps1024", name="psd")
                    ps0, ps1 = psd[:, 0:512], psd[:, 512:1024]
                    for ec in range(NM):
                        nc.tensor.matmul(
                            ps0, wo_sb[ec][:, ts(2 * jtp, 128)],
                            ctxT[ec][:, ts(q4, 512)],
                            start=(ec == 0), stop=(ec == NM - 1))
                        nc.tensor.matmul(
                            ps1, wo_sb[ec][:, ts(2 * jtp + 1, 128)],
                            ctxT[ec][:, ts(q4, 512)],
                            start=(ec == 0), stop=(ec == NM - 1))
                    for jh, ps in ((0, ps0), (1, ps1)):
                        st = finp.tile([128, 512], F32, tag="fin", name="st")
                        nc.vector.tensor_copy(st[:, :], ps)
                        nc.sync.dma_start(
                            fT[ts(2 * jtp + jh, 128), ts(q4, 512)], st[:, :])

            def normalize_block(j, rs_j):
                rec = rbp.tile([72, 512], F32, tag="rec", name="rec", bufs=2)
                nc.vector.reciprocal(rec[64:72, :], rs_j[64:72, :])
                recb = rbp.tile([72, 512], BF, tag="recb", name="recb", bufs=2)
                nc.vector.tensor_copy(recb[64:72, :], rec[64:72, :])
                for hpp in range(2):
                    psn = psp.tile([128, 1024], F32, tag="ps1024", name="psn")
                    for hh in range(2):
                        hp = 2 * hpp + hh
                        nc.tensor.matmul(
                            psn[:, ts(hh, 512)],
                            sel_sb[64:72, ts(hp, 128)], recb[64:72, :],
                            start=True, stop=True, tile_position=(64, 0))
                        nc.vector.tensor_mul(
                            ctxT[hp][:, ts(j, 512)], ctxT[hp][:, ts(j, 512)],
                            psn[:, ts(hh, 512)])

            # ---- software pipeline over token column blocks ----------------
            for n in range(NJ):
                proj_qk(n, xq, wq_sb, qT, qb_t, 0.125)
                proj_qk(n, xk, wk_sb, kT, kb_t, 1.0)
                proj_v(n)
                rs_j = rbp.tile([72, 512], F32, tag="rs", name=f"rs{n}", bufs=2)
                nc.vector.memset(rs_j[64:72, :], 0.0)
                for hp in range(NM):
                    attention_block(hp, n, rs_j)
                normalize_block(n, rs_j)
                oproj_block(n)

    nc.compile()
    return nc


def _get_nc():
    global _NC
    if _NC is None:
        _NC = _build()
    return _NC


def build_in_maps(inputs):
    query = np.asarray(inputs["query"], np.float32)
    key = np.asarray(inputs["key"], np.float32)
    value = np.asarray(inputs["value"], np.float32)
    q_w = np.asarray(inputs["q_w"], np.float32)
    q_b = np.asarray(inputs["q_b"], np.float32)
    k_w = np.asarray(inputs["k_w"], np.float32)
    k_b = np.asarray(inputs["k_b"], np.float32)
    v_w = np.asarray(inputs["v_w"], np.float32)
    o_w = np.asarray(inputs["o_w"], np.float32)

    xqT = [np.ascontiguousarray(query[b].T).astype(BF16) for b in range(B)]
    xkT = [np.ascontiguousarray(key[b].T).astype(BF16) for b in range(B)]
    xvT = [np.ascontiguousarray(value[b].T).astype(BF16) for b in range(B)]

    wqT, wkT, wvT, woT, qbt, kbt = [], [], [], [], [], []
    for g in range(2):
        gs = slice(g * G, (g + 1) * G)
        wqT.append(np.ascontiguousarray(q_w[gs, :].T).astype(BF16))
        wkT.append(np.ascontiguousarray(k_w[gs, :].T).astype(BF16))
        wvT.append(np.ascontiguousarray(v_w[gs, :].T).astype(BF16))
        woT.append(np.ascontiguousarray(o_w[:, gs].T).astype(BF16))
        qbt.append(
            np.ascontiguousarray((q_b[gs] / 8.0).reshape(NM, 128).T).astype(
                np.float32
            )
        )
        kbt.append(
            np.ascontiguousarray(k_b[gs].reshape(NM, 128).T).astype(np.float32)
        )

    sel_np = np.zeros((8, G), np.float32)
    for k in range(8):
        for p in range(G):
            hp, pp = p // 128, p % 128
            if k == 2 * hp + (pp // 64):
                sel_np[k, p] = 1.0
    sel_np = sel_np.astype(BF16)

    in_maps = []
    for b in range(B):
        for g in range(2):
            in_maps.append(
                {
                    "xq": xqT[b],
                    "xk": xkT[b],
                    "xv": xvT[b],
                    "wq": wqT[g],
                    "wk": wkT[g],
                    "wv": wvT[g],
                    "wo": woT[g],
                    "qb": qbt[g],
                    "kb": kbt[g],
                    "sel": sel_np,
                }
            )

    return in_maps


def kernel(**inputs):
    nc = _get_nc()
    in_maps = build_in_maps(inputs)
    res = bass_utils.run_bass_kernel_spmd(nc, in_maps, core_ids=list(range(8)))

    o_b = np.asarray(inputs["o_b"], np.float32)
    v_b = np.asarray(inputs["v_b"], np.float32)
    o_w = np.asarray(inputs["o_w"], np.float32)
    corr = (o_b + v_b @ o_w.T).astype(np.float32)  # softmax rows sum to 1
    out = np.empty((B, S, E), np.float32)
    for b in range(B):
        acc = res.results[2 * b]["fT"] + res.results[2 * b + 1]["fT"]
        out[b] = acc.T + corr[None, :]
    return out
